# revision 14
# baseline (speedup 1.0000x reference)
"""Trainium2 Bass kernel for the AnalyticalBoundedLineAttractor problem.

Reference semantics (per step, per sample):
    z = x @ W.T + b;  m = (z > 0);  A = diag(m) @ W - I;  c = m * b
    x_next = expm(A*dt) @ x + (expm(A*dt) - I) @ pinv(A) @ c

Approximation (validated vs the fp32 jax reference at rel err 9.2e-4,
gate 2e-2): with lam = exp(-dt), a K=2 Taylor expansion of the augmented
matrix exponential gives
    v1 = lam*relu(dt*z);  v2 = m * ((dt/2) W v1 + lam (dt^2/2) b)
    x_next = lam*x + v1 + v2
Two further O(dt^3)-or-smaller rewrites collapse the whole step to ONE
matmul-accumulation plus ONE relu:
  * v1_{t+1} is used in place of v1 inside v2 (the Y increment proxy),
    so the v2 inner product folds into the SAME matmul that computes z.
  * m*(...) + relu(z) == relu(z + ...) except where the tiny O(dt^2)
    term flips the sign of z (rare, O(dt^2) error on that set).
Device recursion (Y_t = x_{t+1} - lam*x_t is the state increment):
    Z'_{t+1} = [MM_x] lam^2 dt W @ x_t
             + [MM_Y] (lam dt + dt/2) W @ Y_t + (lam dt + lam dt^2/2) b
    Y_{t+1}  = relu(Z'_{t+1})            (DVE tensor_scalar, PSUM -> SBUF)
    x_{t+2}  = lam*x_{t+1} + Y_{t+1}     (DVE STT, trajectory record)

Per-step critical cycle: relu (DVE) -> MM_Y (PE) -> relu.  Everything
else (MM_x over the already-recorded x_t, the trajectory STT, chunked
trajectory DMA-out) runs in the slack.  2 matmuls + 2 DVE ops per step.

Sharding: data-parallel over batch, 256/8 = 32 samples per NeuronCore.
The 100-step fp16 trajectory stays in SBUF; chunked DMA out; host casts.
"""

import math
import sys

import numpy as np

try:
    from concourse.bass_utils import run_bass_kernel_spmd
except ImportError:
    sys.path.insert(0, "/opt/trn_rl_repo")
    from concourse.bass_utils import run_bass_kernel_spmd

import concourse.bacc as bacc
import concourse.mybir as mybir
import concourse.tile as tile

DT = 0.05
T_STEPS = 100
DIM = 64
BATCH = 256
N_CORES = 8
BL = BATCH // N_CORES  # 32 samples per core
LAM = math.exp(-DT)
F32 = mybir.dt.float32
F16 = mybir.dt.float16

# wts layout (fp16, [DIM+1, 160]):
#   cols   0: 64  S_x = lam^2 dt W^T                  (rows 0:64 used, K=64)
#   cols  64:128  S_y = [(lam dt + dt/2) W^T ; (lam dt + lam dt^2/2) b^T]
#   cols 128:160  x0 shard scaled by lam dt/(lam dt + dt/2), row 64 = 1
# (the init matmul reuses S_y over the pre-scaled x0, so S_y's W-part and
#  bias row reproduce Z'_0 = lam dt (W x0 + b) + lam dt^2/2 b exactly)
WTS_COLS = 160
X0_SCALE = LAM * DT / (LAM * DT + DT / 2)

_CACHE = {}


def _build_nc():
    nc = bacc.Bacc(None, target_bir_lowering=False)
    wts_ext = nc.declare_dram_parameter("wth", [DIM + 1, WTS_COLS], F16, isOutput=False)
    out_ext = nc.declare_dram_parameter("out", [DIM, T_STEPS * BL], F16, isOutput=True)

    OP = mybir.AluOpType

    with tile.TileContext(nc) as tc:
        with (
            tc.tile_pool(name="sb", bufs=1) as sb,
            tc.tile_pool(name="zp", bufs=3, space="PSUM") as zp,
        ):
            traj = sb.tile([DIM, T_STEPS * BL], F16)
            Y = sb.tile([DIM + 1, 2 * BL], F16)  # two alternating column halves
            wts = sb.tile([DIM + 1, WTS_COLS], F16)

            # critical columns (S_y + x0, gates MM_0) first, S_x second
            nc.sync.dma_start(wts[:, 64:160], wts_ext[:, 64:160])
            nc.sync.dma_start(wts[:, 0:64], wts_ext[:, 0:64])
            nc.vector.memset(Y[DIM : DIM + 1, :], 1.0)

            # init: Z'_0 = lam dt (W x0 + b) + lam dt^2/2 b  (S_y over scaled x0)
            zb_prev = zp.tile([DIM, BL], F32)
            nc.tensor.matmul(
                zb_prev[:], wts[:, 64:128], wts[:, 128:160], start=True, stop=True
            )
            # traj slice 0 = x0 (undo the host pre-scale on the Act engine)
            nc.scalar.mul(traj[:, 0:BL], wts[0:DIM, 128:160], 1.0 / X0_SCALE)

            bounds = {18: 0, 38: 20, 58: 40, 78: 60, 95: 80, 97: 97}
            for t in range(T_STEPS - 1):
                h = (t % 2) * BL
                yfull = Y[:, h : h + BL]
                ydat = Y[0:DIM, h : h + BL]

                # Y_t = relu(Z'_t)   (critical)
                nc.vector.tensor_scalar_max(ydat, zb_prev[:], 0.0)
                if t < T_STEPS - 2:
                    # Z'_{t+1} accumulation (skipped on the last step: unused)
                    zb_cur = zp.tile([DIM, BL], F32)
                    nc.tensor.matmul(
                        zb_cur[:], wts[0:DIM, 0:64], traj[:, t * BL : (t + 1) * BL],
                        start=True, stop=False,
                    )
                    nc.tensor.matmul(
                        zb_cur[:], wts[:, 64:128], yfull, start=False, stop=True
                    )
                else:
                    zb_cur = zb_prev
                if t < T_STEPS - 2:
                    # x_{t+1} = lam x_t + Y_t  (trajectory record, off path)
                    nc.vector.scalar_tensor_tensor(
                        traj[:, (t + 1) * BL : (t + 2) * BL],
                        traj[:, t * BL : (t + 1) * BL],
                        LAM, ydat, op0=OP.mult, op1=OP.add,
                    )
                else:
                    # last step: ship Y_98 raw; host forms x_99 = lam x_98 + Y_98
                    nc.sync.dma_start(
                        out_ext[:, (t + 1) * BL : (t + 2) * BL], ydat
                    )
                zb_prev = zb_cur

                if t in bounds:
                    lo, hi = bounds[t] * BL, (t + 2) * BL
                    nc.sync.dma_start(out_ext[:, lo:hi], traj[:, lo:hi])

    nc.compile()
    return nc


def _host_weights(W, b):
    W64 = W.astype(np.float64)
    b64 = b.astype(np.float64)
    wts = np.zeros((DIM + 1, WTS_COLS), np.float64)
    wts[0:DIM, 0:64] = (LAM * LAM * DT) * W64.T
    wts[0:DIM, 64:128] = (LAM * DT + DT / 2) * W64.T
    wts[DIM, 64:128] = (LAM * DT + LAM * DT * DT / 2) * b64
    wts[DIM, 128:160] = 1.0
    return wts


def _run_device(x0, W, b, **spmd_kwargs):
    if "nc" not in _CACHE:
        _CACHE["nc"] = _build_nc()
    nc = _CACHE["nc"]

    wts = _host_weights(W, b)
    in_maps = []
    for i in range(N_CORES):
        shard = x0[i * BL : (i + 1) * BL].T.astype(np.float64)  # (DIM, BL)
        w = wts.copy()
        w[0:DIM, 128:160] = X0_SCALE * shard
        in_maps.append({"wth": np.ascontiguousarray(w.astype(np.float16))})

    return run_bass_kernel_spmd(
        nc, in_maps, core_ids=list(range(N_CORES)), **spmd_kwargs
    )


def kernel(initial_position, W, b):
    x0 = np.asarray(initial_position, np.float32)
    W = np.asarray(W, np.float32)
    b = np.asarray(b, np.float32)

    res = _run_device(x0, W, b)

    out = np.empty((BATCH, T_STEPS, DIM), np.float32)
    for i in range(N_CORES):
        core_out = res.results[i]["out"].astype(np.float32)  # (DIM, T*BL)
        tr = core_out.reshape(DIM, T_STEPS, BL).transpose(2, 1, 0)  # (BL, T, D)
        # device ships Y_98 in the last slot; form x_99 = lam*x_98 + Y_98
        tr[:, T_STEPS - 1] = LAM * tr[:, T_STEPS - 2] + tr[:, T_STEPS - 1]
        out[i * BL : (i + 1) * BL] = tr
    return out


# revision 15
# speedup vs baseline: 1.0057x; 1.0057x over previous
"""Trainium2 Bass kernel for the AnalyticalBoundedLineAttractor problem.

Reference semantics (per step, per sample):
    z = x @ W.T + b;  m = (z > 0);  A = diag(m) @ W - I;  c = m * b
    x_next = expm(A*dt) @ x + (expm(A*dt) - I) @ pinv(A) @ c

Approximation (validated vs the fp32 jax reference at rel err 9.2e-4,
gate 2e-2): with lam = exp(-dt), a K=2 Taylor expansion of the augmented
matrix exponential gives
    v1 = lam*relu(dt*z);  v2 = m * ((dt/2) W v1 + lam (dt^2/2) b)
    x_next = lam*x + v1 + v2
Two further O(dt^3)-or-smaller rewrites collapse the whole step to ONE
matmul-accumulation plus ONE relu:
  * v1_{t+1} is used in place of v1 inside v2 (the Y increment proxy),
    so the v2 inner product folds into the SAME matmul that computes z.
  * m*(...) + relu(z) == relu(z + ...) except where the tiny O(dt^2)
    term flips the sign of z (rare, O(dt^2) error on that set).
Device recursion (Y_t = x_{t+1} - lam*x_t is the state increment):
    Z'_{t+1} = [MM_x] lam^2 dt W @ x_t
             + [MM_Y] (lam dt + dt/2) W @ Y_t + (lam dt + lam dt^2/2) b
    Y_{t+1}  = relu(Z'_{t+1})            (DVE tensor_scalar, PSUM -> SBUF)
    x_{t+2}  = lam*x_{t+1} + Y_{t+1}     (DVE STT, trajectory record)

Per-step critical cycle: relu (DVE) -> MM_Y (PE) -> relu.  Everything
else (MM_x over the already-recorded x_t, the trajectory STT, chunked
trajectory DMA-out) runs in the slack.  2 matmuls + 2 DVE ops per step.

Sharding: data-parallel over batch, 256/8 = 32 samples per NeuronCore.
The 100-step fp16 trajectory stays in SBUF; chunked DMA out; host casts.
"""

import math
import sys

import numpy as np

try:
    from concourse.bass_utils import run_bass_kernel_spmd
except ImportError:
    sys.path.insert(0, "/opt/trn_rl_repo")
    from concourse.bass_utils import run_bass_kernel_spmd

import concourse.bacc as bacc
import concourse.mybir as mybir
import concourse.tile as tile

DT = 0.05
T_STEPS = 100
DIM = 64
BATCH = 256
N_CORES = 8
BL = BATCH // N_CORES  # 32 samples per core
LAM = math.exp(-DT)
F32 = mybir.dt.float32
F16 = mybir.dt.float16

# wts layout (fp16, [DIM+1, 160]):
#   cols   0: 64  S_x = lam^2 dt W^T                  (rows 0:64 used, K=64)
#   cols  64:128  S_y = [(lam dt + dt/2) W^T ; (lam dt + lam dt^2/2) b^T]
#   cols 128:160  x0 shard scaled by lam dt/(lam dt + dt/2), row 64 = 1
# (the init matmul reuses S_y over the pre-scaled x0, so S_y's W-part and
#  bias row reproduce Z'_0 = lam dt (W x0 + b) + lam dt^2/2 b exactly)
WTS_COLS = 160
X0_SCALE = LAM * DT / (LAM * DT + DT / 2)

_CACHE = {}


def _build_nc():
    nc = bacc.Bacc(None, target_bir_lowering=False)
    wts_ext = nc.declare_dram_parameter("wth", [DIM + 1, WTS_COLS], F16, isOutput=False)
    out_ext = nc.declare_dram_parameter("out", [DIM, T_STEPS * BL], F16, isOutput=True)

    OP = mybir.AluOpType

    with tile.TileContext(nc) as tc:
        with (
            tc.tile_pool(name="sb", bufs=1) as sb,
            tc.tile_pool(name="zp", bufs=3, space="PSUM") as zp,
        ):
            traj = sb.tile([DIM, T_STEPS * BL], F16)
            Y = sb.tile([DIM + 1, 2 * BL], F16)  # two alternating column halves
            wts = sb.tile([DIM + 1, WTS_COLS], F16)

            nc.gpsimd.dma_start(wts[:], wts_ext[:])
            nc.vector.memset(Y[DIM : DIM + 1, :], 1.0)

            # init: Z'_0 = lam dt (W x0 + b) + lam dt^2/2 b  (S_y over scaled x0)
            zb_prev = zp.tile([DIM, BL], F32)
            nc.tensor.matmul(
                zb_prev[:], wts[:, 64:128], wts[:, 128:160], start=True, stop=True
            )
            # traj slice 0 = x0 (undo the host pre-scale on the Act engine)
            nc.scalar.mul(traj[:, 0:BL], wts[0:DIM, 128:160], 1.0 / X0_SCALE)

            bounds = {18: 0, 38: 20, 58: 40, 78: 60, 95: 80, 97: 97}
            for t in range(T_STEPS - 1):
                h = (t % 2) * BL
                yfull = Y[:, h : h + BL]
                ydat = Y[0:DIM, h : h + BL]

                # Y_t = relu(Z'_t)   (critical)
                nc.vector.tensor_scalar_max(ydat, zb_prev[:], 0.0)
                if t < T_STEPS - 2:
                    # Z'_{t+1} accumulation (skipped on the last step: unused)
                    zb_cur = zp.tile([DIM, BL], F32)
                    nc.tensor.matmul(
                        zb_cur[:], wts[0:DIM, 0:64], traj[:, t * BL : (t + 1) * BL],
                        start=True, stop=False,
                    )
                    nc.tensor.matmul(
                        zb_cur[:], wts[:, 64:128], yfull, start=False, stop=True
                    )
                else:
                    zb_cur = zb_prev
                if t < T_STEPS - 2:
                    # x_{t+1} = lam x_t + Y_t  (trajectory record, off path)
                    nc.vector.scalar_tensor_tensor(
                        traj[:, (t + 1) * BL : (t + 2) * BL],
                        traj[:, t * BL : (t + 1) * BL],
                        LAM, ydat, op0=OP.mult, op1=OP.add,
                    )
                else:
                    # last step: ship Y_98 raw; host forms x_99 = lam x_98 + Y_98
                    nc.sync.dma_start(
                        out_ext[:, (t + 1) * BL : (t + 2) * BL], ydat
                    )
                zb_prev = zb_cur

                if t in bounds:
                    lo, hi = bounds[t] * BL, (t + 2) * BL
                    nc.sync.dma_start(out_ext[:, lo:hi], traj[:, lo:hi])

    nc.compile()
    return nc


def _host_weights(W, b):
    W64 = W.astype(np.float64)
    b64 = b.astype(np.float64)
    wts = np.zeros((DIM + 1, WTS_COLS), np.float64)
    wts[0:DIM, 0:64] = (LAM * LAM * DT) * W64.T
    wts[0:DIM, 64:128] = (LAM * DT + DT / 2) * W64.T
    wts[DIM, 64:128] = (LAM * DT + LAM * DT * DT / 2) * b64
    wts[DIM, 128:160] = 1.0
    return wts


def _run_device(x0, W, b, **spmd_kwargs):
    if "nc" not in _CACHE:
        _CACHE["nc"] = _build_nc()
    nc = _CACHE["nc"]

    wts = _host_weights(W, b)
    in_maps = []
    for i in range(N_CORES):
        shard = x0[i * BL : (i + 1) * BL].T.astype(np.float64)  # (DIM, BL)
        w = wts.copy()
        w[0:DIM, 128:160] = X0_SCALE * shard
        in_maps.append({"wth": np.ascontiguousarray(w.astype(np.float16))})

    return run_bass_kernel_spmd(
        nc, in_maps, core_ids=list(range(N_CORES)), **spmd_kwargs
    )


def kernel(initial_position, W, b):
    x0 = np.asarray(initial_position, np.float32)
    W = np.asarray(W, np.float32)
    b = np.asarray(b, np.float32)

    res = _run_device(x0, W, b)

    out = np.empty((BATCH, T_STEPS, DIM), np.float32)
    for i in range(N_CORES):
        core_out = res.results[i]["out"].astype(np.float32)  # (DIM, T*BL)
        tr = core_out.reshape(DIM, T_STEPS, BL).transpose(2, 1, 0)  # (BL, T, D)
        # device ships Y_98 in the last slot; form x_99 = lam*x_98 + Y_98
        tr[:, T_STEPS - 1] = LAM * tr[:, T_STEPS - 2] + tr[:, T_STEPS - 1]
        out[i * BL : (i + 1) * BL] = tr
    return out


# revision 16
# speedup vs baseline: 1.0067x; 1.0010x over previous
"""Trainium2 Bass kernel for the AnalyticalBoundedLineAttractor problem.

Reference semantics (per step, per sample):
    z = x @ W.T + b;  m = (z > 0);  A = diag(m) @ W - I;  c = m * b
    x_next = expm(A*dt) @ x + (expm(A*dt) - I) @ pinv(A) @ c

Approximation (validated vs the fp32 jax reference at rel err 9.2e-4,
gate 2e-2): with lam = exp(-dt), a K=2 Taylor expansion of the augmented
matrix exponential gives
    v1 = lam*relu(dt*z);  v2 = m * ((dt/2) W v1 + lam (dt^2/2) b)
    x_next = lam*x + v1 + v2
Two further O(dt^3)-or-smaller rewrites collapse the whole step to ONE
matmul-accumulation plus ONE relu:
  * v1_{t+1} is used in place of v1 inside v2 (the Y increment proxy),
    so the v2 inner product folds into the SAME matmul that computes z.
  * m*(...) + relu(z) == relu(z + ...) except where the tiny O(dt^2)
    term flips the sign of z (rare, O(dt^2) error on that set).
Device recursion (Y_t = x_{t+1} - lam*x_t is the state increment):
    Z'_{t+1} = [MM_x] lam^2 dt W @ x_t
             + [MM_Y] (lam dt + dt/2) W @ Y_t + (lam dt + lam dt^2/2) b
    Y_{t+1}  = relu(Z'_{t+1})            (DVE tensor_scalar, PSUM -> SBUF)
    x_{t+2}  = lam*x_{t+1} + Y_{t+1}     (DVE STT, trajectory record)

Per-step critical cycle: relu (DVE) -> MM_Y (PE) -> relu.  Everything
else (MM_x over the already-recorded x_t, the trajectory STT, chunked
trajectory DMA-out) runs in the slack.  2 matmuls + 2 DVE ops per step.

Sharding: data-parallel over batch, 256/8 = 32 samples per NeuronCore.
The 100-step fp16 trajectory stays in SBUF; chunked DMA out; host casts.
"""

import math
import sys

import numpy as np

try:
    from concourse.bass_utils import run_bass_kernel_spmd
except ImportError:
    sys.path.insert(0, "/opt/trn_rl_repo")
    from concourse.bass_utils import run_bass_kernel_spmd

import concourse.bacc as bacc
import concourse.mybir as mybir
import concourse.tile as tile

DT = 0.05
T_STEPS = 100
DIM = 64
BATCH = 256
N_CORES = 8
BL = BATCH // N_CORES  # 32 samples per core
LAM = math.exp(-DT)
F32 = mybir.dt.float32
F16 = mybir.dt.float16

# wts layout (fp16, [DIM+1, 160]):
#   cols   0: 64  S_x = lam^2 dt W^T                  (rows 0:64 used, K=64)
#   cols  64:128  S_y = [(lam dt + dt/2) W^T ; (lam dt + lam dt^2/2) b^T]
#   cols 128:160  x0 shard scaled by lam dt/(lam dt + dt/2), row 64 = 1
# (the init matmul reuses S_y over the pre-scaled x0, so S_y's W-part and
#  bias row reproduce Z'_0 = lam dt (W x0 + b) + lam dt^2/2 b exactly)
WTS_COLS = 160
X0_SCALE = LAM * DT / (LAM * DT + DT / 2)

_CACHE = {}


def _build_nc():
    nc = bacc.Bacc(None, target_bir_lowering=False)
    wts_ext = nc.declare_dram_parameter("wth", [DIM + 1, WTS_COLS], F16, isOutput=False)
    out_ext = nc.declare_dram_parameter("out", [DIM, T_STEPS * BL], F16, isOutput=True)

    OP = mybir.AluOpType

    with tile.TileContext(nc) as tc:
        with (
            tc.tile_pool(name="sb", bufs=1) as sb,
            tc.tile_pool(name="zp", bufs=3, space="PSUM") as zp,
        ):
            traj = sb.tile([DIM, T_STEPS * BL], F16)
            Y = sb.tile([DIM + 1, 2 * BL], F16)  # two alternating column halves
            wts = sb.tile([DIM + 1, WTS_COLS], F16)

            nc.sync.dma_start(wts[:], wts_ext[:])
            nc.vector.memset(Y[DIM : DIM + 1, :], 1.0)

            # init: Z'_0 = lam dt (W x0 + b) + lam dt^2/2 b  (S_y over scaled x0)
            zb_prev = zp.tile([DIM, BL], F32)
            nc.tensor.matmul(
                zb_prev[:], wts[:, 64:128], wts[:, 128:160], start=True, stop=True
            )
            # traj slice 0 = x0 (undo the host pre-scale on the Act engine)
            nc.scalar.mul(traj[:, 0:BL], wts[0:DIM, 128:160], 1.0 / X0_SCALE)

            bounds = {18: 0, 38: 20, 58: 40, 78: 60, 95: 80, 97: 97}
            for t in range(T_STEPS - 1):
                h = (t % 2) * BL
                yfull = Y[:, h : h + BL]
                ydat = Y[0:DIM, h : h + BL]

                # Y_t = relu(Z'_t)   (critical)
                nc.vector.tensor_scalar_max(ydat, zb_prev[:], 0.0)
                if t < T_STEPS - 2:
                    # Z'_{t+1} accumulation (skipped on the last step: unused)
                    zb_cur = zp.tile([DIM, BL], F32)
                    nc.tensor.matmul(
                        zb_cur[:], wts[0:DIM, 0:64], traj[:, t * BL : (t + 1) * BL],
                        start=True, stop=False,
                    )
                    nc.tensor.matmul(
                        zb_cur[:], wts[:, 64:128], yfull, start=False, stop=True
                    )
                else:
                    zb_cur = zb_prev
                if t < T_STEPS - 2:
                    # x_{t+1} = lam x_t + Y_t  (trajectory record, off path)
                    nc.vector.scalar_tensor_tensor(
                        traj[:, (t + 1) * BL : (t + 2) * BL],
                        traj[:, t * BL : (t + 1) * BL],
                        LAM, ydat, op0=OP.mult, op1=OP.add,
                    )
                else:
                    # last step: ship Y_98 raw; host forms x_99 = lam x_98 + Y_98
                    nc.sync.dma_start(
                        out_ext[:, (t + 1) * BL : (t + 2) * BL], ydat
                    )
                zb_prev = zb_cur

                if t in bounds:
                    lo, hi = bounds[t] * BL, (t + 2) * BL
                    nc.sync.dma_start(out_ext[:, lo:hi], traj[:, lo:hi])

    nc.compile()
    return nc


def _host_weights(W, b):
    W64 = W.astype(np.float64)
    b64 = b.astype(np.float64)
    wts = np.zeros((DIM + 1, WTS_COLS), np.float64)
    wts[0:DIM, 0:64] = (LAM * LAM * DT) * W64.T
    wts[0:DIM, 64:128] = (LAM * DT + DT / 2) * W64.T
    wts[DIM, 64:128] = (LAM * DT + LAM * DT * DT / 2) * b64
    wts[DIM, 128:160] = 1.0
    return wts


def _run_device(x0, W, b, **spmd_kwargs):
    if "nc" not in _CACHE:
        _CACHE["nc"] = _build_nc()
    nc = _CACHE["nc"]

    wts = _host_weights(W, b)
    in_maps = []
    for i in range(N_CORES):
        shard = x0[i * BL : (i + 1) * BL].T.astype(np.float64)  # (DIM, BL)
        w = wts.copy()
        w[0:DIM, 128:160] = X0_SCALE * shard
        in_maps.append({"wth": np.ascontiguousarray(w.astype(np.float16))})

    return run_bass_kernel_spmd(
        nc, in_maps, core_ids=list(range(N_CORES)), **spmd_kwargs
    )


def kernel(initial_position, W, b):
    x0 = np.asarray(initial_position, np.float32)
    W = np.asarray(W, np.float32)
    b = np.asarray(b, np.float32)

    res = _run_device(x0, W, b)

    out = np.empty((BATCH, T_STEPS, DIM), np.float32)
    for i in range(N_CORES):
        core_out = res.results[i]["out"].astype(np.float32)  # (DIM, T*BL)
        tr = core_out.reshape(DIM, T_STEPS, BL).transpose(2, 1, 0)  # (BL, T, D)
        # device ships Y_98 in the last slot; form x_99 = lam*x_98 + Y_98
        tr[:, T_STEPS - 1] = LAM * tr[:, T_STEPS - 2] + tr[:, T_STEPS - 1]
        out[i * BL : (i + 1) * BL] = tr
    return out


# revision 17
# speedup vs baseline: 1.0169x; 1.0102x over previous
"""Trainium2 Bass kernel for the AnalyticalBoundedLineAttractor problem.

Reference semantics (per step, per sample):
    z = x @ W.T + b;  m = (z > 0);  A = diag(m) @ W - I;  c = m * b
    x_next = expm(A*dt) @ x + (expm(A*dt) - I) @ pinv(A) @ c

Approximation (validated vs the fp32 jax reference at rel err 9.2e-4,
gate 2e-2): with lam = exp(-dt), a K=2 Taylor expansion of the augmented
matrix exponential gives
    v1 = lam*relu(dt*z);  v2 = m * ((dt/2) W v1 + lam (dt^2/2) b)
    x_next = lam*x + v1 + v2
Two further O(dt^3)-or-smaller rewrites collapse the whole step to ONE
matmul-accumulation plus ONE relu:
  * v1_{t+1} is used in place of v1 inside v2 (the Y increment proxy),
    so the v2 inner product folds into the SAME matmul that computes z.
  * m*(...) + relu(z) == relu(z + ...) except where the tiny O(dt^2)
    term flips the sign of z (rare, O(dt^2) error on that set).
Device recursion (Y_t = x_{t+1} - lam*x_t is the state increment):
    Z'_{t+1} = [MM_x] lam^2 dt W @ x_t
             + [MM_Y] (lam dt + dt/2) W @ Y_t + (lam dt + lam dt^2/2) b
    Y_{t+1}  = relu(Z'_{t+1})            (DVE tensor_scalar, PSUM -> SBUF)
    x_{t+2}  = lam*x_{t+1} + Y_{t+1}     (DVE STT, trajectory record)

Per-step critical cycle: relu (DVE) -> MM_Y (PE) -> relu.  Everything
else (MM_x over the already-recorded x_t, the trajectory STT, chunked
trajectory DMA-out) runs in the slack.  2 matmuls + 2 DVE ops per step.

Sharding: data-parallel over batch, 256/8 = 32 samples per NeuronCore.
The 100-step fp16 trajectory stays in SBUF; chunked DMA out; host casts.
"""

import math
import sys

import numpy as np

try:
    from concourse.bass_utils import run_bass_kernel_spmd
except ImportError:
    sys.path.insert(0, "/opt/trn_rl_repo")
    from concourse.bass_utils import run_bass_kernel_spmd

import concourse.bacc as bacc
import concourse.mybir as mybir
import concourse.tile as tile

DT = 0.05
T_STEPS = 100
DIM = 64
BATCH = 256
N_CORES = 8
BL = BATCH // N_CORES  # 32 samples per core
LAM = math.exp(-DT)
F32 = mybir.dt.float32
F16 = mybir.dt.float16

# wts layout (fp16, [DIM+1, 160]):
#   cols   0: 64  S_x = lam^2 dt W^T                  (rows 0:64 used, K=64)
#   cols  64:128  S_y = [(lam dt + dt/2) W^T ; (lam dt + lam dt^2/2) b^T]
#   cols 128:160  x0 shard scaled by lam dt/(lam dt + dt/2), row 64 = 1
# (the init matmul reuses S_y over the pre-scaled x0, so S_y's W-part and
#  bias row reproduce Z'_0 = lam dt (W x0 + b) + lam dt^2/2 b exactly)
WTS_COLS = 160
X0_SCALE = LAM * DT / (LAM * DT + DT / 2)

_CACHE = {}


def _build_nc():
    nc = bacc.Bacc(None, target_bir_lowering=False)
    wts_ext = nc.declare_dram_parameter("wth", [DIM + 1, WTS_COLS], F16, isOutput=False)
    out_ext = nc.declare_dram_parameter("out", [DIM, T_STEPS * BL], F16, isOutput=True)

    OP = mybir.AluOpType

    with tile.TileContext(nc) as tc:
        with (
            tc.tile_pool(name="sb", bufs=1) as sb,
            tc.tile_pool(name="zp", bufs=3, space="PSUM") as zp,
        ):
            traj = sb.tile([DIM, T_STEPS * BL], F16)
            Y = sb.tile([DIM + 1, 2 * BL], F16)  # two alternating column halves
            wts = sb.tile([DIM + 1, WTS_COLS], F16)

            nc.sync.dma_start(wts[:], wts_ext[:])
            nc.vector.memset(Y[DIM : DIM + 1, :], 1.0)

            # init: Z'_0 = lam dt (W x0 + b) + lam dt^2/2 b  (S_y over scaled x0)
            zb_prev = zp.tile([DIM, BL], F32)
            nc.tensor.matmul(
                zb_prev[:], wts[:, 64:128], wts[:, 128:160], start=True, stop=True
            )
            # traj slice 0 = x0 (undo the host pre-scale; vector is idle here
            # and using Act would trigger a 1.3us ACT_TABLE_LOAD at startup)
            nc.vector.tensor_scalar_mul(
                traj[:, 0:BL], wts[0:DIM, 128:160], 1.0 / X0_SCALE
            )

            bounds = {18: 0, 38: 20, 58: 40, 78: 60, 95: 80, 97: 97}
            for t in range(T_STEPS - 1):
                h = (t % 2) * BL
                yfull = Y[:, h : h + BL]
                ydat = Y[0:DIM, h : h + BL]

                # Y_t = relu(Z'_t)   (critical)
                nc.vector.tensor_scalar_max(ydat, zb_prev[:], 0.0)
                if t < T_STEPS - 2:
                    # Z'_{t+1} accumulation (skipped on the last step: unused)
                    zb_cur = zp.tile([DIM, BL], F32)
                    nc.tensor.matmul(
                        zb_cur[:], wts[0:DIM, 0:64], traj[:, t * BL : (t + 1) * BL],
                        start=True, stop=False,
                    )
                    nc.tensor.matmul(
                        zb_cur[:], wts[:, 64:128], yfull, start=False, stop=True
                    )
                else:
                    zb_cur = zb_prev
                if t < T_STEPS - 2:
                    # x_{t+1} = lam x_t + Y_t  (trajectory record, off path)
                    nc.vector.scalar_tensor_tensor(
                        traj[:, (t + 1) * BL : (t + 2) * BL],
                        traj[:, t * BL : (t + 1) * BL],
                        LAM, ydat, op0=OP.mult, op1=OP.add,
                    )
                else:
                    # last step: ship Y_98 raw; host forms x_99 = lam x_98 + Y_98
                    nc.sync.dma_start(
                        out_ext[:, (t + 1) * BL : (t + 2) * BL], ydat
                    )
                zb_prev = zb_cur

                if t in bounds:
                    lo, hi = bounds[t] * BL, (t + 2) * BL
                    nc.sync.dma_start(out_ext[:, lo:hi], traj[:, lo:hi])

    nc.compile()
    return nc


def _host_weights(W, b):
    W64 = W.astype(np.float64)
    b64 = b.astype(np.float64)
    wts = np.zeros((DIM + 1, WTS_COLS), np.float64)
    wts[0:DIM, 0:64] = (LAM * LAM * DT) * W64.T
    wts[0:DIM, 64:128] = (LAM * DT + DT / 2) * W64.T
    wts[DIM, 64:128] = (LAM * DT + LAM * DT * DT / 2) * b64
    wts[DIM, 128:160] = 1.0
    return wts


def _run_device(x0, W, b, **spmd_kwargs):
    if "nc" not in _CACHE:
        _CACHE["nc"] = _build_nc()
    nc = _CACHE["nc"]

    wts = _host_weights(W, b)
    in_maps = []
    for i in range(N_CORES):
        shard = x0[i * BL : (i + 1) * BL].T.astype(np.float64)  # (DIM, BL)
        w = wts.copy()
        w[0:DIM, 128:160] = X0_SCALE * shard
        in_maps.append({"wth": np.ascontiguousarray(w.astype(np.float16))})

    return run_bass_kernel_spmd(
        nc, in_maps, core_ids=list(range(N_CORES)), **spmd_kwargs
    )


def kernel(initial_position, W, b):
    x0 = np.asarray(initial_position, np.float32)
    W = np.asarray(W, np.float32)
    b = np.asarray(b, np.float32)

    res = _run_device(x0, W, b)

    out = np.empty((BATCH, T_STEPS, DIM), np.float32)
    for i in range(N_CORES):
        core_out = res.results[i]["out"].astype(np.float32)  # (DIM, T*BL)
        tr = core_out.reshape(DIM, T_STEPS, BL).transpose(2, 1, 0)  # (BL, T, D)
        # device ships Y_98 in the last slot; form x_99 = lam*x_98 + Y_98
        tr[:, T_STEPS - 1] = LAM * tr[:, T_STEPS - 2] + tr[:, T_STEPS - 1]
        out[i * BL : (i + 1) * BL] = tr
    return out
